# revision 11
# baseline (speedup 1.0000x reference)
"""Trainium2 Bass kernel for the DQC1 data-reuploading circuit — spectral method.

Math: f(x) = Re(<00| W_L prod_k S_k(x) W_k |00>) is an analytic, band-limited
function of the scalar x: every layer's diagonal contributes phases e^{i(+-theta)x},
so f's spectrum lives in [-Omega, Omega] with Omega = sum_k max(theta_k0, theta_k1)
(~11.8 here). On the clipped input range [-6, 6] f is therefore captured exactly
by a J=31-term Fourier series with period T=16 (grid spacing 2*pi/16 resolves the
interval, J*2*pi/16 = 12.2 > Omega). The host fits the 63 coefficients by least
squares against the exact recurrence evaluated on a 4001-point grid (O(grid *
DEGREE) work, independent of N); the fit reproduces the reference to ~1e-13.

Device pipeline per core (32768 points as 16384 columns, 2 points/column;
63 basis rows per point: rows 0..62 = point A, 64..126 = point B):
  1. PE broadcast:   u[p,f] = scale_p * x_f            (lhsT [2,128]: xA,xB rows)
  2. round:          i = round_nearest_int32(u + b_p)  (ScalarE Identity / DVE ts)
  3. subtract:       w = u - i                         (DVE TT, exact in fp32)
  4. Sin:            basis = sin(2pi*w + 2pi*b_p) = sin(2pi*(u+b_p))  (fp16 out)
  5. PE contraction: f = coef . basis, accumulating 16 slices into one [32,512]
     PSUM tile via per-slice stationaries that are zero except rows (k, 16+k).
A PE warmup burst runs during the input-DMA window to lift the HAM clock gate;
input DMAs are split across engine queues to parallelize the transfer.
Data parallel across 8 NeuronCores; x sharded, constants replicated.
"""

import sys

sys.path.insert(0, "/opt/trn_rl_repo")

import numpy as np

import concourse.bass as bass
import concourse.bacc as bacc
import concourse.tile as tile
from concourse import mybir
from concourse.bass_utils import run_bass_kernel_spmd

N_CORES = 8
DEGREE = 20
P = 128
XMAX = 6.0
T_PER = 16.0
J = 31
NROW = 2 * J + 1  # 63 basis rows per point
F32 = mybir.dt.float32
F32R = mybir.dt.float32r
F16 = mybir.dt.float16
I32 = mybir.dt.int32
AF = mybir.ActivationFunctionType
OP = mybir.AluOpType
TWO_PI = 2.0 * float(np.pi)

SE_ROUND = 10  # of 16 round ops issued on ScalarE (rest on DVE); trace-tuned
N_WARM = 14  # PE warmup matmuls during the input-DMA window


def _forward_host(x, theta, phi):
    """Exact reference forward in float64 for a vector of x values."""
    theta = np.asarray(theta, np.float64)
    phi = np.asarray(phi, np.float64)

    def rx(t):
        c, s = np.cos(t / 2), np.sin(t / 2)
        return np.array([[c, -1j * s], [-1j * s, c]])

    def ry(t):
        c, s = np.cos(t / 2), np.sin(t / 2)
        return np.array([[c, -s], [s, c]])

    def rz(t):
        e = np.exp(-0.5j * t)
        return np.array([[e, 0], [0, np.conj(e)]])

    def w_layer(p):
        A = rz(p[2]) @ ry(p[1]) @ rx(p[0])
        B = rz(p[5]) @ ry(p[4]) @ rx(p[3])
        M = np.kron(A, B)
        M[3, :] *= -1.0
        return M

    W = [w_layer(phi[k]) for k in range(DEGREE + 1)]
    n = x.shape[0]
    U = np.broadcast_to(np.eye(4, dtype=complex), (n, 4, 4)).copy()
    for k in range(DEGREE):
        c0, s0 = np.cos(theta[k, 0] * x / 2), np.sin(theta[k, 0] * x / 2)
        c1, s1 = np.cos(theta[k, 1] * x / 2), np.sin(theta[k, 1] * x / 2)
        a = np.zeros((n, 2, 2), complex)
        a[:, 0, 0] = c0
        a[:, 0, 1] = -1j * s0
        a[:, 1, 0] = -1j * s0
        a[:, 1, 1] = c0
        b = np.zeros((n, 2, 2), complex)
        b[:, 0, 0] = c1
        b[:, 0, 1] = -1j * s1
        b[:, 1, 0] = -1j * s1
        b[:, 1, 1] = c1
        S = np.einsum("nij,npq->nipjq", a, b).reshape(n, 4, 4)
        U = np.einsum("nij,njk->nik", S, W[k][None] @ U)
    U = W[DEGREE][None] @ U
    return np.real(U[:, 0, 0])


def _host_constants(theta, phi):
    """Fit the Fourier coefficients and build the device constant tables."""
    xg = np.linspace(-XMAX, XMAX, 4001)
    fg = _forward_host(xg, theta, phi)
    nu = np.arange(J + 1) / T_PER  # cycles per unit x
    A = np.concatenate(
        [np.cos(TWO_PI * np.outer(xg, nu)), np.sin(TWO_PI * np.outer(xg, nu[1:]))],
        axis=1,
    )
    wgt = np.exp(-(xg**2) / 4)
    coef, *_ = np.linalg.lstsq(A * wgt[:, None], fg * wgt, rcond=None)

    # basis row r (within a 63-row block): r<=J -> cos j=r (bias .25); else sin j=r-J
    scales = np.concatenate([nu, nu[1:]])
    biases = np.concatenate([0.25 * np.ones(J + 1), np.zeros(J)])

    bc = np.zeros((2, P), np.float32)  # lhsT: u = scale * x_{A|B}
    bc[0, 0:NROW] = scales
    bc[1, 64 : 64 + NROW] = scales

    scl = np.zeros((P, 2), np.float32)  # col0: b_p (round bias); col1: 2*pi*b_p
    scl[0:NROW, 0] = biases
    scl[64 : 64 + NROW, 0] = biases
    scl[:, 1] = TWO_PI * scl[:, 0]

    # contraction stationaries: slice position k (0..15) writes psum rows k, 16+k
    cot = np.zeros((P, 16, 32), np.float16)
    for k in range(16):
        cot[0:NROW, k, k] = coef
        cot[64 : 64 + NROW, k, 16 + k] = coef
    return {"bc": bc, "scl": scl, "cot": cot.reshape(P, 512)}


def build_program(B):
    """Bass program for one core processing B points (B = 32768)."""
    H = B // 2  # 16384 columns, 2 points per column
    nc = bacc.Bacc("TRN2", target_bir_lowering=False, debug=False)

    xm_d = nc.declare_dram_parameter("xm", [2, H], F32R, isOutput=False)
    bc_d = nc.declare_dram_parameter("bc", [2, P], F32R, isOutput=False)
    scl_d = nc.declare_dram_parameter("scl", [P, 2], F32, isOutput=False)
    cot_d = nc.declare_dram_parameter("cot", [P, 512], F16, isOutput=False)
    out_d = nc.declare_dram_parameter("out", [B], F32, isOutput=True)
    # out layout: O[16r + k, 512g + c] = f[r*16384 + g*8192 + k*512 + c]

    from contextlib import ExitStack

    with ExitStack() as ctx:
        tc = ctx.enter_context(tile.TileContext(nc))
        const = ctx.enter_context(tc.tile_pool(name="const", bufs=1))
        ubp = ctx.enter_context(tc.tile_pool(name="ub", bufs=3, space="PSUM"))
        pop = ctx.enter_context(tc.tile_pool(name="po", bufs=2, space="PSUM"))
        ip = ctx.enter_context(tc.tile_pool(name="i32", bufs=3))
        wp = ctx.enter_context(tc.tile_pool(name="w", bufs=2))
        bp = ctx.enter_context(tc.tile_pool(name="basis", bufs=2))
        op_ = ctx.enter_context(tc.tile_pool(name="o", bufs=1))

        # --- PE warmup: lift the HAM clock gate while input DMAs stream.
        warm = const.tile([P, 512], F32, tag="warm")
        nc.vector.memset(warm[:], 0.0)
        for _ in range(N_WARM):
            wpo = pop.tile([32, 512], F32, tag="po")
            nc.tensor.matmul(wpo[:], warm[:, 0:32], warm[:], start=True, stop=True)

        # --- constants; xm split across engine DMA queues to parallelize.
        xm = const.tile([2, H], F32R, tag="xm")
        nc.sync.dma_start(xm[0:1, :], xm_d[0:1, :])
        nc.gpsimd.dma_start(xm[1:2, 0 : H // 2], xm_d[1:2, 0 : H // 2])
        nc.scalar.dma_start(xm[1:2, H // 2 : H], xm_d[1:2, H // 2 : H])
        bc = const.tile([2, P], F32R, tag="bc")
        nc.sync.dma_start(bc[:], bc_d[:, :])
        scl = const.tile([P, 2], F32, tag="scl")
        nc.sync.dma_start(scl[:], scl_d[:, :])
        cot = const.tile([P, 512], F16, tag="cot")
        nc.sync.dma_start(cot[:], cot_d[:, :])
        bC = scl[:, 0:1]
        b2pi = scl[:, 1:2]

        # tiny Sin first so the trig_and_small act table loads before the hot loop
        warm2 = const.tile([P, 8], F32, tag="warm2")
        nc.scalar.activation(warm2[:], warm[:, 0:8], AF.Sin, scale=TWO_PI)

        O = op_.tile([32, 1024], F32, tag="o")
        po = None
        wbig = None
        basis = None
        pending = []

        for s2 in range(16):  # ub-tile index; covers slices 2*s2, 2*s2+1
            ub = ubp.tile([P, 1024], F32, tag="ub")
            for h in range(2):
                s = 2 * s2 + h
                nc.tensor.matmul(
                    ub[:, 512 * h : 512 * (h + 1)],
                    bc[:],
                    xm[:, 512 * s : 512 * (s + 1)],
                    start=True,
                    stop=True,
                )
            i32 = ip.tile([P, 1024], I32, tag="i32")
            if ((s2 + 1) * SE_ROUND) // 16 > (s2 * SE_ROUND) // 16:
                nc.scalar.activation(i32[:], ub[:], AF.Identity, bias=bC)
            else:
                nc.vector.tensor_scalar(i32[:], ub[:], bC, None, OP.add)
            if s2 % 2 == 0:
                wbig = wp.tile([P, 2048], F32, tag="w")
            q = 1024 * (s2 % 2)
            nc.vector.tensor_tensor(wbig[:, q : q + 1024], ub[:], i32[:], OP.subtract)
            pending.append(s2)
            if s2 % 2 == 1:
                basis = bp.tile([P, 2048], F16, tag="basis")
                nc.scalar.activation(
                    basis[:], wbig[:], AF.Sin, bias=b2pi, scale=TWO_PI
                )
                for t2 in pending:
                    for h in range(2):
                        s = 2 * t2 + h
                        k = s % 16
                        if k == 0:
                            po = pop.tile([32, 512], F32, tag="po")
                        nc.tensor.matmul(
                            po[:],
                            cot[:, 32 * k : 32 * k + 32],
                            basis[:, 512 * (s % 4) : 512 * (s % 4 + 1)],
                            start=(k == 0),
                            stop=(k == 15),
                        )
                        if k == 15:
                            g = s // 16
                            nc.scalar.activation(
                                O[:, 512 * g : 512 * (g + 1)], po[:], AF.Copy
                            )
                            for r in range(2):
                                dst = out_d[
                                    r * 16384 + g * 8192 : r * 16384 + (g + 1) * 8192
                                ]
                                nc.sync.dma_start(
                                    dst.rearrange("(k c) -> k c", c=512),
                                    O[16 * r : 16 * r + 16, 512 * g : 512 * (g + 1)],
                                )
                pending = []

    nc.compile()
    return nc


_CACHE = {}


def _get_program(B):
    if B not in _CACHE:
        _CACHE[B] = build_program(B)
    return _CACHE[B]


def run(data_point, theta, phi, trace=False):
    data_point = np.ascontiguousarray(np.asarray(data_point, np.float32))
    n = data_point.shape[0]
    B = n // N_CORES
    consts = _host_constants(theta, phi)
    nc = _get_program(B)
    shards = np.clip(data_point.reshape(N_CORES, B), -XMAX, XMAX)
    in_maps = [
        dict(consts, xm=np.ascontiguousarray(shards[i].reshape(2, B // 2)))
        for i in range(N_CORES)
    ]
    res = run_bass_kernel_spmd(nc, in_maps, list(range(N_CORES)), trace=trace)
    out = np.concatenate([np.asarray(res.results[i]["out"]) for i in range(N_CORES)])
    return out, res


def kernel(data_point, theta, phi):
    out, _ = run(data_point, theta, phi)
    return out
